# revision 44
# baseline (speedup 1.0000x reference)
"""Trainium2 Bass kernel for cross-attention with per-head structured mask.

Reference computation (B=4, N=1024, DIM=1024, H=16, D=64):
    q = x1 @ Wq;  k, v = split(x2 @ Wkv)
    dots = q k^T * D^-0.5 + spd
    attn = softmax(dots) * (head_keep * H / n_kept)   # whole heads dropped
    out  = (attn @ v) @ Wo + bo

Sharding: dropped heads contribute exactly zero, so only kept heads are
computed. Work unit = (batch b, kept-head group g): 8 cores = 4 batches x 2
head groups. Each core computes a partial out[b] (its heads' contribution
through Wo); host sums the two partials per batch and adds the bias.

Device layout (per core, H_c heads):
    QT[hd, n], KT[hd, m] via PE (contraction over DIM, inputs pre-transposed
    on host).  V held as [m, h, 128] blocks: per head, 64 cols of V plus 64
    cols of ones (parity-swapped), so ctx_psum = V_aug^T @ exp(scores^T)
    carries both the context rows AND the softmax denominator rows in one
    accumulation, landing at the partition base the final ctxT layout needs.

spd path: the host ships exp(spd) (bf16) instead of spd; since
exp(s + p) = exp(s) * exp(p), the bias enters as a DVE elementwise
multiply on exp(scores) instead of an identity matmul on the PE
(removes ~56 matmuls + 56 LDWEIGHTS, ~14us of PE time, 104.4 -> 91.3us).
Output partials are written bf16 and summed fp32 on host.

HW quirks baked in (found empirically):
  - custom-DVE ops (reciprocal_approx_fast) and K=1 matmul operands only
    behave at partition base 0 -> shuttle rows down via tiny DMAs.
  - fp32/fp32r matmuls run at ~2-4 cyc/row; bf16 runs at 1 cyc/row, so
    matmul operands default to bf16 (PSUM accumulation stays fp32).
  - fp8 (float8e4) fails everywhere: espd in fp8 -> rel err 2.3e-2 (gate
    2e-2) AND a slow DVE tensor_tensor path; fp8 weights blow the error
    budget analytically.
  - engines execute their instruction queues IN PROGRAM ORDER: emitting
    work that depends on late DMAs (e.g. Q-proj of the second n-half)
    before phase B stalls the whole PE queue.  Conversely, outstanding
    DMA descriptors share bandwidth CONCURRENTLY (not FIFO), so issuing
    prefetches early steals bandwidth from urgent loads; spdp bufs=4 is
    the tuned prefetch throttle.
  - per-DMA-descriptor throughput is ~12-25GB/s (packets round-robin all
    16 engines); sustained ingest needs several descriptors in flight.
  - gpsimd software-DGE dma_start is much slower; gpsimd cannot touch
    PSUM at all.
  - dense back-to-back PE streams trigger power throttling (util capped
    ~0.5-0.75 with hysteresis); the device also drifts ~20% slower when
    hot from repeated benching - let it idle a few minutes before
    trusting a measurement.
"""

import os

import numpy as np

B, N, DIM = 4, 1024, 1024
HEADS, DIM_HEAD = 16, 64
INNER = HEADS * DIM_HEAD
SCALE = DIM_HEAD ** -0.5
NCORES = 8
KT = DIM // 128      # 8 contraction tiles
NB = N // 512        # 2 column blocks
MT = N // 128        # 8 key tiles

_cache: dict = {}


def _build(H_c: int, keep_scale: float, mode: str = "bf16", half_last: bool = False):
    """Build + compile the per-core Bass program for H_c heads (H_c even)."""
    import concourse.mybir as mybir
    import concourse.tile as tile
    from concourse import bacc

    dt = mybir.dt
    f32 = dt.float32
    HB = H_c // 2
    HD = H_c * DIM_HEAD
    assert H_c % 2 == 0 and HD <= 512

    mmdt = {"bf16": dt.bfloat16, "f32r": dt.float32r, "f32": f32}[mode]

    nc = bacc.Bacc("TRN2", target_bir_lowering=False)

    xq = nc.dram_tensor("xq", [128, KT, N], mmdt, kind="ExternalInput")   # x1[b].T, sbuf image
    xk = nc.dram_tensor("xk", [128, KT, N], mmdt, kind="ExternalInput")   # x2[b].T, sbuf image
    wq = nc.dram_tensor("wq", [128, KT, HD], mmdt, kind="ExternalInput")  # sbuf layout
    wk = nc.dram_tensor("wk", [128, KT, HD], mmdt, kind="ExternalInput")
    wv = nc.dram_tensor("wv", [128, KT, HD], mmdt, kind="ExternalInput")
    wo = nc.dram_tensor("wo", [128, HD // 128, DIM], mmdt, kind="ExternalInput")
    # exp(spd[b,h]).T in [n-block, partition, m-tile, n] sbuf-image layout;
    # exp(s + p) = exp(s) * exp(p), so the spd bias enters as an elementwise
    # multiply on DVE instead of an identity matmul on the PE.  (fp8 fails
    # by MAX-statistics: full-fp8 2.30e-2, half-fp8 2.16e-2 vs gate 2e-2 —
    # the absmax error rides single worst-case quantization events, so
    # shrinking the fp8 region barely helps; bf16 is the floor.)
    spddt = mmdt
    spd = nc.dram_tensor("spd", [H_c, NB, 128, MT, 512], spddt, kind="ExternalInput")
    # bf16 partials: host sums the two per-batch partials in fp32
    out = nc.dram_tensor("out", [N, DIM], dt.bfloat16, kind="ExternalOutput")

    Exp = mybir.ActivationFunctionType.Exp
    mult = mybir.AluOpType.mult

    with tile.TileContext(nc) as tc:
        with (
            tc.tile_pool(name="w", bufs=1) as wpool,
            tc.tile_pool(name="big", bufs=1) as big,
            tc.tile_pool(name="spdp", bufs=4) as spdp,
            tc.tile_pool(name="work", bufs=6) as work,
            tc.tile_pool(name="psA", bufs=3, space="PSUM") as psA,
            tc.tile_pool(name="psS", bufs=3, space="PSUM") as psS,
            tc.tile_pool(name="psC", bufs=2, space="PSUM") as psC,
        ):
            wq_sb = wpool.tile([128, KT, HD], mmdt, tag="wq")
            wk_sb = wpool.tile([128, KT, HD], mmdt, tag="wk")
            wv_sb = wpool.tile([128, KT, HD], mmdt, tag="wv")
            wo_sb = wpool.tile([128, HD // 128, DIM], mmdt, tag="wo")
            # x2T image goes first, k-chunked and m-halved so the K/V
            # projection of the first 512-col slab starts after ~1.5MB
            # instead of 2.5MB
            # startup fill: few big full-partition descriptors split across
            # the two HWDGE engines (scalar is idle until the first exp).
            # Descriptor-issue bandwidth is scarce: keep the count low so the
            # spd streams are not delayed behind startup issues.
            xk_sb = big.tile([128, KT, N], mmdt, tag="xkim")
            xq_sb = big.tile([128, KT, N], mmdt, tag="xqim")
            nc.sync.dma_start(wk_sb[:, 0, :], wk[:, 0, :])
            nc.scalar.dma_start(xk_sb[:, 0, 0:512], xk[:, 0, 0:512])
            nc.sync.dma_start(wk_sb[:, 1:, :], wk[:, 1:, :])
            nc.scalar.dma_start(wv_sb[:], wv[:])
            for k in range(1, KT):
                eng = nc.sync if k % 2 else nc.scalar
                eng.dma_start(xk_sb[:, k, 0:512], xk[:, k, 0:512])
            for k in range(KT):
                eng = nc.sync if k % 2 else nc.scalar
                eng.dma_start(xk_sb[:, k, 512:N], xk[:, k, 512:N])
            nc.scalar.dma_start(wq_sb[:], wq[:])
            for k in range(KT):
                eng = nc.sync if k % 2 else nc.scalar
                eng.dma_start(xq_sb[:, k, 0:512], xq[:, k, 0:512])
            spd_pre = {}
            for k in range(KT):
                eng = nc.sync if k % 2 else nc.scalar
                eng.dma_start(xq_sb[:, k, 512:N], xq[:, k, 512:N])
            nc.sync.dma_start(wo_sb[:], wo[:])

            qt_sb = big.tile([128, HB, N], mmdt, tag="qt")
            kt_sb = big.tile([128, HB, N], mmdt, tag="kt")
            v_sb = big.tile([128, MT, H_c * 128], mmdt, tag="v")
            ct_sb = big.tile([128, HB, N], mmdt, tag="ct")

            if half_last:
                # the shared head's slot is only computed at local n0=0; its
                # n0=1 region of ctxT must read as zero in the out projection
                nc.gpsimd.memset(ct_sb[64:128, HB - 1, 512:N], 0.0)

            # ones columns of the augmented V blocks (parity-swapped per head)
            for h in range(H_c):
                c0 = h * 128 + (64 if h % 2 == 0 else 0)
                for m in range(MT):
                    nc.gpsimd.memset(v_sb[:, m, c0:c0 + 64], 1.0)

            # ---- Phase A: projections. One 1MB slab DMA per 512-column
            # block; KT and V share the x2T slabs. ----
            def v_copyout(ps_ap, m):
                pv = ps_ap.rearrange("p (hb two d) -> p hb two d", two=2, d=64)
                vv = v_sb[:, m, :].rearrange("p (hb x) -> p hb x", hb=HB)
                # even heads -> value cols 0:64 of their block; odd -> 192:256
                nc.vector.tensor_copy(vv[:, :, 0:64], pv[:, :, 0, :])
                nc.vector.tensor_copy(vv[:, :, 192:256], pv[:, :, 1, :])

            def kv_proj(m0):
                m_sl = slice(m0 * 512, (m0 + 1) * 512)
                slab = xk_sb[:, :, m_sl]
                kps = [psA.tile([128, 512], f32, tag="acc",
                                name=f"kps{m0}_{i}") for i in range(HB)]
                vps = [psS.tile([128, HD], f32, tag="sc",
                                name=f"vps{m0}_{i}") for i in range(2)]
                for k in range(KT):
                    for hb in range(HB):
                        nc.tensor.matmul(
                            kps[hb][:],
                            wk_sb[:, k, hb * 128:(hb + 1) * 128],
                            slab[:, k, :],
                            start=(k == 0), stop=(k == KT - 1),
                        )
                    for mi in range(2):
                        nc.tensor.matmul(
                            vps[mi][:],
                            slab[:, k, mi * 128:(mi + 1) * 128],
                            wv_sb[:, k, :],
                            start=(k == 0), stop=(k == KT - 1),
                        )
                for hb in range(HB):
                    nc.vector.tensor_copy(kt_sb[:, hb, m_sl], kps[hb][:])
                for mi in range(2):
                    v_copyout(vps[mi][:], m0 * 4 + mi)
                # second half of the V m-tiles from the same slab
                vpsb = [psS.tile([128, HD], f32, tag="sc",
                                 name=f"vpsb{m0}_{i}") for i in range(2)]
                for k in range(KT):
                    for mi in range(2):
                        nc.tensor.matmul(
                            vpsb[mi][:],
                            slab[:, k, (2 + mi) * 128:(3 + mi) * 128],
                            wv_sb[:, k, :],
                            start=(k == 0), stop=(k == KT - 1),
                        )
                for mi in range(2):
                    v_copyout(vpsb[mi][:], m0 * 4 + 2 + mi)

            def q_proj(n0):
                n_sl = slice(n0 * 512, (n0 + 1) * 512)
                slab = xq_sb[:, :, n_sl]
                qps = [psA.tile([128, 512], f32, tag="acc",
                                name=f"psq{n0}_{i}") for i in range(HB)]
                for k in range(KT):
                    for hb in range(HB):
                        nc.tensor.matmul(
                            qps[hb][:],
                            wq_sb[:, k, hb * 128:(hb + 1) * 128],
                            slab[:, k, :],
                            start=(k == 0), stop=(k == KT - 1),
                        )
                for hb in range(HB):
                    nc.vector.tensor_copy(qt_sb[:, hb, n_sl], qps[hb][:])

            # ---- Phase A ----
            kv_proj(0)
            kv_proj(1)
            q_proj(0)
            q_proj(1)

            # ---- Phase B (attention) + C (out proj), interleaved per n-block ----
            for n0 in range(NB):
                n_sl = slice(n0 * 512, (n0 + 1) * 512)
                nheads = H_c - 1 if (half_last and n0 > 0) else H_c
                # even-parity heads first, odd last: the final unit's sumexp
                # already sits at partition base 0, skipping the recip shuttle
                # copy on the tail-critical path
                horder = [h for h in range(nheads) if h % 2 == 0] + \
                         [h for h in range(nheads) if h % 2 == 1]
                for h in horder:
                    hb, hp = divmod(h, 2)
                    vb = hp * 64          # partition base of ctx values
                    sb_ = 64 - vb         # partition base of sumexp rows
                    ctx = psC.tile([128, 512], f32, tag="ctx")
                    if (h, n0) in spd_pre:
                        spds = spd_pre[(h, n0)]
                    else:
                        spds = spdp.tile([128, MT, 512], spddt, tag="spd")
                        nc.sync.dma_start(spds[:, 0:MT // 2, :],
                                          spd[h, n0, :, 0:MT // 2, :])
                        nc.sync.dma_start(spds[:, MT // 2:, :],
                                          spd[h, n0, :, MT // 2:, :])
                    for m in range(MT):
                        sc = psS.tile([128, 512], f32, tag="sc")
                        nc.tensor.matmul(
                            sc[:],
                            kt_sb[vb:vb + 64, hb, m * 128:(m + 1) * 128],
                            qt_sb[vb:vb + 64, hb, n_sl],
                            start=True, stop=True,
                        )
                        # e = exp(qk) * exp(spd): exp on ACT (686ns/tile, the
                        # B-phase pace), bias-multiply on DVE (415ns/tile)
                        es = work.tile([128, 512], mmdt, tag="es")
                        nc.scalar.activation(es[:], sc[:], Exp)
                        e = work.tile([128, 512], mmdt, tag="e")
                        nc.vector.tensor_tensor(
                            e[:], es[:], spds[:, m, :], op=mult)
                        nc.tensor.matmul(
                            ctx[:],
                            v_sb[:, m, h * 128:(h + 1) * 128],
                            e[:],
                            start=(m == 0),
                            stop=(m == MT - 1),
                        )
                    # normalize: ctxT = ctx_vals * keep_scale / sumexp (the
                    # ones block replicated sumexp across 64 rows at base sb_;
                    # copy to base 0 -- custom-DVE recip needs base 0 -- then
                    # stt against the PSUM value rows).
                    rr = work.tile([128, 512], f32, tag="rr")
                    if sb_ == 0:
                        nc.vector.reciprocal_approx_fast(
                            rr[0:64, :], ctx[0:64, :])
                    else:
                        ss = work.tile([128, 512], f32, tag="ss")
                        nc.vector.tensor_copy(ss[0:64, :], ctx[sb_:sb_ + 64, :])
                        nc.vector.reciprocal_approx_fast(rr[0:64, :], ss[0:64, :])
                    nc.vector.scalar_tensor_tensor(
                        out=ct_sb[vb:vb + 64, hb, n_sl],
                        in0=ctx[vb:vb + 64, :],
                        scalar=float(keep_scale),
                        in1=rr[0:64, :],
                        op0=mult,
                        op1=mult,
                    )

                # ---- Phase C for this n-block ----
                # n0=0: copies on DVE, DMAs on sync (hidden under B1).
                # n0=1 (tail): the two copies per row-tile split DVE/scalar in
                # parallel and the writes alternate both HWDGE engines, since
                # scalar is idle after the last exp.
                last = (n0 == NB - 1)
                for nt in range(n0 * 4, (n0 + 1) * 4):
                    ot = work.tile([128, 2, 512], dt.bfloat16, tag="o")
                    for d0 in range(NB):
                        po = psA.tile([128, 512], f32, tag="acc",
                                      name=f"po{nt}_{d0}")
                        for kk in range(HD // 128):
                            nc.tensor.matmul(
                                po[:],
                                ct_sb[:, kk, nt * 128:(nt + 1) * 128],
                                wo_sb[:, kk, d0 * 512:(d0 + 1) * 512],
                                start=(kk == 0),
                                stop=(kk == HD // 128 - 1),
                            )
                        if last and d0 == 1:
                            nc.scalar.copy(ot[:, d0, :], po[:])
                        else:
                            nc.vector.tensor_copy(ot[:, d0, :], po[:])
                        eng = (nc.scalar if (last and d0 == 1) else nc.sync)
                        eng.dma_start(
                            out[nt * 128:(nt + 1) * 128,
                                d0 * 512:(d0 + 1) * 512],
                            ot[:, d0, :])

    nc.finalize()
    return nc


def _get_nc(H_c: int, n_kept: int, mode: str, half_last: bool):
    key = (H_c, n_kept, mode, half_last)
    if key not in _cache:
        _cache[key] = _build(H_c, HEADS / n_kept, mode, half_last)
    return _cache[key]


def _prep_inputs(x1, x2, spd, head_keep, Wq, Wkv, Wo, mode="bf16"):
    """Slice/transpose/pad host-side into per-core input maps."""
    import ml_dtypes

    ndt = np.float32 if mode in ("f32", "f32r") else ml_dtypes.bfloat16
    kept = [int(i) for i in np.nonzero(head_keep)[0]]
    n_kept = len(kept)
    half_last = (n_kept % 2 == 1)
    if not half_last:
        H_c = n_kept // 2
        if H_c % 2:
            H_c += 1
        groups = [kept[:H_c], kept[H_c:]]
    else:
        # odd count: both cores of a pair share the last kept head, each
        # computing one n-half of it (local column order differs per core)
        K = (n_kept - 1) // 2
        shared = kept[-1]
        H_c = K + 1
        pad = []
        if H_c % 2:
            H_c += 1
            pad = [None]
        groups = [kept[:K] + pad + [shared], kept[K:2 * K] + pad + [shared]]

    Wk_full, Wv_full = Wkv[:, :INNER], Wkv[:, INNER:]

    in_maps = []
    for b in range(B):
        xqT = np.ascontiguousarray(
            x1[b].T.reshape(KT, 128, N).transpose(1, 0, 2)).astype(ndt)
        xkT = np.ascontiguousarray(
            x2[b].T.reshape(KT, 128, N).transpose(1, 0, 2)).astype(ndt)
        for g in range(2):
            heads = groups[g]
            swap = half_last and g == 1  # local n0=0 <-> global half 1
            xq_g = xqT
            if swap:
                xq_g = np.ascontiguousarray(
                    np.concatenate([xqT[:, :, 512:], xqT[:, :, :512]], axis=2))
            HD = H_c * DIM_HEAD
            wq_c = np.zeros((DIM, HD), np.float32)
            wk_c = np.zeros((DIM, HD), np.float32)
            wv_c = np.zeros((DIM, HD), np.float32)
            wo_c = np.zeros((HD, DIM), np.float32)
            # exp(spd) multiplies exp(qk); absent slots get 1.0 (identity)
            spd_c = np.ones((H_c, NB, 128, MT, 512), ndt)
            for i, h in enumerate(heads):
                if h is None:
                    continue
                sl = slice(i * DIM_HEAD, (i + 1) * DIM_HEAD)
                hs = slice(h * DIM_HEAD, (h + 1) * DIM_HEAD)
                wq_c[:, sl] = Wq[:, hs] * SCALE
                wk_c[:, sl] = Wk_full[:, hs]
                wv_c[:, sl] = Wv_full[:, hs]
                wo_c[sl, :] = Wo[hs, :]
                # exp(spd[b,h]).T -> [n-block, partition, m-tile, n] image,
                # n-blocks in the core's LOCAL column order
                im = (np.exp(spd[b, h].T).reshape(MT, 128, NB, 512)
                      .transpose(2, 1, 0, 3)).astype(ndt)
                spd_c[i] = im[::-1] if swap else im
            in_maps.append({
                "xq": xq_g,
                "xk": xkT,
                "wq": np.ascontiguousarray(
                    wq_c.reshape(KT, 128, HD).transpose(1, 0, 2)).astype(ndt),
                "wk": np.ascontiguousarray(
                    wk_c.reshape(KT, 128, HD).transpose(1, 0, 2)).astype(ndt),
                "wv": np.ascontiguousarray(
                    wv_c.reshape(KT, 128, HD).transpose(1, 0, 2)).astype(ndt),
                "wo": np.ascontiguousarray(
                    wo_c.reshape(HD // 128, 128, DIM).transpose(1, 0, 2)).astype(ndt),
                "spd": spd_c,
            })
    return in_maps, n_kept, H_c


def _run(nc, in_maps, trace=False, tmpdir=None):
    from concourse.bass_utils import run_bass_kernel_spmd

    return run_bass_kernel_spmd(
        nc, in_maps, core_ids=list(range(NCORES)), trace=trace, tmpdir=tmpdir
    )


def kernel(x1, x2, spd, head_keep, Wq, Wkv, Wo, bo, _trace=False, _tmpdir=None):
    x1 = np.asarray(x1, np.float32)
    x2 = np.asarray(x2, np.float32)
    spd = np.asarray(spd, np.float32)
    head_keep = np.asarray(head_keep)
    n_kept = int(head_keep.astype(np.int64).sum())
    if n_kept == 0:
        # reference: 16/0 = inf, 0*inf = nan everywhere
        return np.full((B, N, DIM), np.nan, np.float32)

    mode = os.environ.get("KERNEL_DTYPE", "bf16")
    in_maps, n_kept, H_c = _prep_inputs(
        x1, x2, spd, head_keep, Wq, Wkv, Wo, mode)
    half_last = (n_kept % 2 == 1)
    nc = _get_nc(H_c, n_kept, mode, half_last)
    res = _run(nc, in_maps, trace=_trace, tmpdir=_tmpdir)

    out = np.empty((B, N, DIM), np.float32)
    bo32 = np.asarray(bo, np.float32)
    for b in range(B):
        o0 = res.results[2 * b]["out"].astype(np.float32)
        o1 = res.results[2 * b + 1]["out"].astype(np.float32)
        if half_last:
            o1 = np.concatenate([o1[512:], o1[:512]], axis=0)
        out[b] = o0 + o1 + bo32
    kernel._last_results = res
    return out



# revision 45
# speedup vs baseline: 1.1717x; 1.1717x over previous
"""Trainium2 Bass kernel for cross-attention with per-head structured mask.

Reference computation (B=4, N=1024, DIM=1024, H=16, D=64):
    q = x1 @ Wq;  k, v = split(x2 @ Wkv)
    dots = q k^T * D^-0.5 + spd
    attn = softmax(dots) * (head_keep * H / n_kept)   # whole heads dropped
    out  = (attn @ v) @ Wo + bo

Sharding: dropped heads contribute exactly zero, so only kept heads are
computed. Work unit = (batch b, kept-head group g): 8 cores = 4 batches x 2
head groups. Each core computes a partial out[b] (its heads' contribution
through Wo); host sums the two partials per batch and adds the bias.

Device layout (per core, H_c heads):
    QT[hd, n], KT[hd, m] via PE (contraction over DIM, inputs pre-transposed
    on host).  V held as [m, h, 128] blocks: per head, 64 cols of V plus 64
    cols of ones (parity-swapped), so ctx_psum = V_aug^T @ exp(scores^T)
    carries both the context rows AND the softmax denominator rows in one
    accumulation, landing at the partition base the final ctxT layout needs.

spd path: the host ships exp(spd) (bf16) instead of spd; since
exp(s + p) = exp(s) * exp(p), the bias enters as a DVE elementwise
multiply on exp(scores) instead of an identity matmul on the PE
(removes ~56 matmuls + 56 LDWEIGHTS, ~14us of PE time, 104.4 -> 91.3us).
Output partials are written bf16 and summed fp32 on host.

HW quirks baked in (found empirically):
  - custom-DVE ops (reciprocal_approx_fast) and K=1 matmul operands only
    behave at partition base 0 -> shuttle rows down via tiny DMAs.
  - fp32/fp32r matmuls run at ~2-4 cyc/row; bf16 runs at 1 cyc/row, so
    matmul operands default to bf16 (PSUM accumulation stays fp32).
  - fp8 (float8e4) fails everywhere: espd in fp8 -> rel err 2.3e-2 (gate
    2e-2) AND a slow DVE tensor_tensor path; fp8 weights blow the error
    budget analytically.
  - engines execute their instruction queues IN PROGRAM ORDER: emitting
    work that depends on late DMAs (e.g. Q-proj of the second n-half)
    before phase B stalls the whole PE queue.  Conversely, outstanding
    DMA descriptors share bandwidth CONCURRENTLY (not FIFO), so issuing
    prefetches early steals bandwidth from urgent loads; spdp bufs=4 is
    the tuned prefetch throttle.
  - per-DMA-descriptor throughput is ~12-25GB/s (packets round-robin all
    16 engines); sustained ingest needs several descriptors in flight.
  - gpsimd software-DGE dma_start is much slower; gpsimd cannot touch
    PSUM at all.
  - dense back-to-back PE streams trigger power throttling (util capped
    ~0.5-0.75 with hysteresis); the device also drifts ~20% slower when
    hot from repeated benching - let it idle a few minutes before
    trusting a measurement.
"""

import os

import numpy as np

B, N, DIM = 4, 1024, 1024
HEADS, DIM_HEAD = 16, 64
INNER = HEADS * DIM_HEAD
SCALE = DIM_HEAD ** -0.5
NCORES = 8
KT = DIM // 128      # 8 contraction tiles
NB = N // 512        # 2 column blocks
MT = N // 128        # 8 key tiles

_cache: dict = {}


def _build(H_c: int, keep_scale: float, mode: str = "bf16", half_last: bool = False):
    """Build + compile the per-core Bass program for H_c heads (H_c even)."""
    import concourse.mybir as mybir
    import concourse.tile as tile
    from concourse import bacc

    dt = mybir.dt
    f32 = dt.float32
    HB = H_c // 2
    HD = H_c * DIM_HEAD
    assert H_c % 2 == 0 and HD <= 512

    mmdt = {"bf16": dt.bfloat16, "f32r": dt.float32r, "f32": f32}[mode]

    nc = bacc.Bacc("TRN2", target_bir_lowering=False)

    xq = nc.dram_tensor("xq", [128, KT, N], mmdt, kind="ExternalInput")   # x1[b].T, sbuf image
    xk = nc.dram_tensor("xk", [128, KT, N], mmdt, kind="ExternalInput")   # x2[b].T, sbuf image
    wq = nc.dram_tensor("wq", [128, KT, HD], mmdt, kind="ExternalInput")  # sbuf layout
    wk = nc.dram_tensor("wk", [128, KT, HD], mmdt, kind="ExternalInput")
    wv = nc.dram_tensor("wv", [128, KT, HD], mmdt, kind="ExternalInput")
    wo = nc.dram_tensor("wo", [128, HD // 128, DIM], mmdt, kind="ExternalInput")
    # exp(spd[b,h]).T in [n-block, partition, m-tile, n] sbuf-image layout;
    # exp(s + p) = exp(s) * exp(p), so the spd bias enters as an elementwise
    # multiply on DVE instead of an identity matmul on the PE.  (fp8 fails
    # by MAX-statistics: full-fp8 2.30e-2, half-fp8 2.16e-2 vs gate 2e-2 —
    # the absmax error rides single worst-case quantization events, so
    # shrinking the fp8 region barely helps; bf16 is the floor.)
    spddt = mmdt
    spd = nc.dram_tensor("spd", [H_c, NB, 128, MT, 512], spddt, kind="ExternalInput")
    # bf16 partials: host sums the two per-batch partials in fp32
    out = nc.dram_tensor("out", [N, DIM], dt.bfloat16, kind="ExternalOutput")

    Exp = mybir.ActivationFunctionType.Exp
    mult = mybir.AluOpType.mult

    with tile.TileContext(nc) as tc:
        with (
            tc.tile_pool(name="w", bufs=1) as wpool,
            tc.tile_pool(name="big", bufs=1) as big,
            tc.tile_pool(name="spdp", bufs=4) as spdp,
            tc.tile_pool(name="work", bufs=6) as work,
            tc.tile_pool(name="psA", bufs=3, space="PSUM") as psA,
            tc.tile_pool(name="psS", bufs=3, space="PSUM") as psS,
            tc.tile_pool(name="psC", bufs=2, space="PSUM") as psC,
        ):
            wq_sb = wpool.tile([128, KT, HD], mmdt, tag="wq")
            wk_sb = wpool.tile([128, KT, HD], mmdt, tag="wk")
            wv_sb = wpool.tile([128, KT, HD], mmdt, tag="wv")
            wo_sb = wpool.tile([128, HD // 128, DIM], mmdt, tag="wo")
            # x2T image goes first, k-chunked and m-halved so the K/V
            # projection of the first 512-col slab starts after ~1.5MB
            # instead of 2.5MB
            # startup fill: few big full-partition descriptors split across
            # the two HWDGE engines (scalar is idle until the first exp).
            # Descriptor-issue bandwidth is scarce: keep the count low so the
            # spd streams are not delayed behind startup issues.
            xk_sb = big.tile([128, KT, N], mmdt, tag="xkim")
            xq_sb = big.tile([128, KT, N], mmdt, tag="xqim")
            nc.sync.dma_start(wk_sb[:, 0, :], wk[:, 0, :])
            nc.scalar.dma_start(xk_sb[:, 0, 0:512], xk[:, 0, 0:512])
            nc.sync.dma_start(wk_sb[:, 1:, :], wk[:, 1:, :])
            nc.scalar.dma_start(wv_sb[:], wv[:])
            for k in range(1, KT):
                eng = nc.sync if k % 2 else nc.scalar
                eng.dma_start(xk_sb[:, k, 0:512], xk[:, k, 0:512])
            for k in range(KT):
                eng = nc.sync if k % 2 else nc.scalar
                eng.dma_start(xk_sb[:, k, 512:N], xk[:, k, 512:N])
            nc.scalar.dma_start(wq_sb[:], wq[:])
            for k in range(KT):
                eng = nc.sync if k % 2 else nc.scalar
                eng.dma_start(xq_sb[:, k, 0:512], xq[:, k, 0:512])
            spd_pre = {}
            for k in range(KT):
                eng = nc.sync if k % 2 else nc.scalar
                eng.dma_start(xq_sb[:, k, 512:N], xq[:, k, 512:N])
            nc.sync.dma_start(wo_sb[:], wo[:])

            qt_sb = big.tile([128, HB, N], mmdt, tag="qt")
            kt_sb = big.tile([128, HB, N], mmdt, tag="kt")
            v_sb = big.tile([128, MT, H_c * 128], mmdt, tag="v")
            ct_sb = big.tile([128, HB, N], mmdt, tag="ct")

            if half_last:
                # the shared head's slot is only computed at local n0=0; its
                # n0=1 region of ctxT must read as zero in the out projection
                nc.gpsimd.memset(ct_sb[64:128, HB - 1, 512:N], 0.0)

            # ones columns of the augmented V blocks (parity-swapped per head)
            for h in range(H_c):
                c0 = h * 128 + (64 if h % 2 == 0 else 0)
                for m in range(MT):
                    nc.gpsimd.memset(v_sb[:, m, c0:c0 + 64], 1.0)

            # ---- Phase A: projections. One 1MB slab DMA per 512-column
            # block; KT and V share the x2T slabs. ----
            def v_copyout(ps_ap, m):
                pv = ps_ap.rearrange("p (hb two d) -> p hb two d", two=2, d=64)
                vv = v_sb[:, m, :].rearrange("p (hb x) -> p hb x", hb=HB)
                # even heads -> value cols 0:64 of their block; odd -> 192:256
                nc.vector.tensor_copy(vv[:, :, 0:64], pv[:, :, 0, :])
                nc.vector.tensor_copy(vv[:, :, 192:256], pv[:, :, 1, :])

            def kv_proj(m0):
                m_sl = slice(m0 * 512, (m0 + 1) * 512)
                slab = xk_sb[:, :, m_sl]
                kps = [psA.tile([128, 512], f32, tag="acc",
                                name=f"kps{m0}_{i}") for i in range(HB)]
                vps = [psS.tile([128, HD], f32, tag="sc",
                                name=f"vps{m0}_{i}") for i in range(2)]
                for k in range(KT):
                    for hb in range(HB):
                        nc.tensor.matmul(
                            kps[hb][:],
                            wk_sb[:, k, hb * 128:(hb + 1) * 128],
                            slab[:, k, :],
                            start=(k == 0), stop=(k == KT - 1),
                        )
                    for mi in range(2):
                        nc.tensor.matmul(
                            vps[mi][:],
                            slab[:, k, mi * 128:(mi + 1) * 128],
                            wv_sb[:, k, :],
                            start=(k == 0), stop=(k == KT - 1),
                        )
                for hb in range(HB):
                    nc.vector.tensor_copy(kt_sb[:, hb, m_sl], kps[hb][:])
                for mi in range(2):
                    v_copyout(vps[mi][:], m0 * 4 + mi)
                # second half of the V m-tiles from the same slab
                vpsb = [psS.tile([128, HD], f32, tag="sc",
                                 name=f"vpsb{m0}_{i}") for i in range(2)]
                for k in range(KT):
                    for mi in range(2):
                        nc.tensor.matmul(
                            vpsb[mi][:],
                            slab[:, k, (2 + mi) * 128:(3 + mi) * 128],
                            wv_sb[:, k, :],
                            start=(k == 0), stop=(k == KT - 1),
                        )
                for mi in range(2):
                    v_copyout(vpsb[mi][:], m0 * 4 + 2 + mi)

            def q_proj(n0):
                n_sl = slice(n0 * 512, (n0 + 1) * 512)
                slab = xq_sb[:, :, n_sl]
                qps = [psA.tile([128, 512], f32, tag="acc",
                                name=f"psq{n0}_{i}") for i in range(HB)]
                for k in range(KT):
                    for hb in range(HB):
                        nc.tensor.matmul(
                            qps[hb][:],
                            wq_sb[:, k, hb * 128:(hb + 1) * 128],
                            slab[:, k, :],
                            start=(k == 0), stop=(k == KT - 1),
                        )
                for hb in range(HB):
                    nc.vector.tensor_copy(qt_sb[:, hb, n_sl], qps[hb][:])

            # ---- Phase A ----
            kv_proj(0)
            kv_proj(1)
            q_proj(0)
            q_proj(1)

            # ---- Phase B (attention) + C (out proj), interleaved per n-block ----
            for n0 in range(NB):
                n_sl = slice(n0 * 512, (n0 + 1) * 512)
                nheads = H_c - 1 if (half_last and n0 > 0) else H_c
                # even-parity heads first, odd last: the final unit's sumexp
                # already sits at partition base 0, skipping the recip shuttle
                # copy on the tail-critical path
                horder = [h for h in range(nheads) if h % 2 == 0] + \
                         [h for h in range(nheads) if h % 2 == 1]
                for h in horder:
                    hb, hp = divmod(h, 2)
                    vb = hp * 64          # partition base of ctx values
                    sb_ = 64 - vb         # partition base of sumexp rows
                    ctx = psC.tile([128, 512], f32, tag="ctx")
                    if (h, n0) in spd_pre:
                        spds = spd_pre[(h, n0)]
                    else:
                        spds = spdp.tile([128, MT, 512], spddt, tag="spd")
                        nc.sync.dma_start(spds[:, 0:MT // 2, :],
                                          spd[h, n0, :, 0:MT // 2, :])
                        nc.sync.dma_start(spds[:, MT // 2:, :],
                                          spd[h, n0, :, MT // 2:, :])
                    # software-pipelined m-loop, depth 2: the PE queue is
                    # in-order, so without a prologue a late spd stalls
                    # ctx(m) AND blocks sc(m+1) behind it, starving the exp
                    # stream.  Emitting sc0,sc1,sc2,ctx0,sc3,ctx1,... keeps
                    # a ~2-tile exp buffer across spd hiccups (3 psS banks).
                    scs = {}

                    def emit_sc(m):
                        sc = psS.tile([128, 512], f32, tag="sc")
                        nc.tensor.matmul(
                            sc[:],
                            kt_sb[vb:vb + 64, hb, m * 128:(m + 1) * 128],
                            qt_sb[vb:vb + 64, hb, n_sl],
                            start=True, stop=True,
                        )
                        scs[m] = sc

                    def emit_rest(m):
                        # e = exp(qk) * exp(spd): exp on ACT (686ns/tile, the
                        # B-phase pace), bias-multiply on DVE (415ns/tile)
                        sc = scs.pop(m)
                        es = work.tile([128, 512], mmdt, tag="es")
                        nc.scalar.activation(es[:], sc[:], Exp)
                        e = work.tile([128, 512], mmdt, tag="e")
                        nc.vector.tensor_tensor(
                            e[:], es[:], spds[:, m, :], op=mult)
                        nc.tensor.matmul(
                            ctx[:],
                            v_sb[:, m, h * 128:(h + 1) * 128],
                            e[:],
                            start=(m == 0),
                            stop=(m == MT - 1),
                        )

                    PD = 2
                    for m in range(MT + PD):
                        if m < MT:
                            emit_sc(m)
                        if m >= PD:
                            emit_rest(m - PD)
                    # normalize: ctxT = ctx_vals * keep_scale / sumexp (the
                    # ones block replicated sumexp across 64 rows at base sb_;
                    # copy to base 0 -- custom-DVE recip needs base 0 -- then
                    # stt against the PSUM value rows).
                    rr = work.tile([128, 512], f32, tag="rr")
                    if sb_ == 0:
                        nc.vector.reciprocal_approx_fast(
                            rr[0:64, :], ctx[0:64, :])
                    else:
                        ss = work.tile([128, 512], f32, tag="ss")
                        nc.vector.tensor_copy(ss[0:64, :], ctx[sb_:sb_ + 64, :])
                        nc.vector.reciprocal_approx_fast(rr[0:64, :], ss[0:64, :])
                    nc.vector.scalar_tensor_tensor(
                        out=ct_sb[vb:vb + 64, hb, n_sl],
                        in0=ctx[vb:vb + 64, :],
                        scalar=float(keep_scale),
                        in1=rr[0:64, :],
                        op0=mult,
                        op1=mult,
                    )

                # ---- Phase C for this n-block ----
                # n0=0: copies on DVE, DMAs on sync (hidden under B1).
                # n0=1 (tail): the two copies per row-tile split DVE/scalar in
                # parallel and the writes alternate both HWDGE engines, since
                # scalar is idle after the last exp.
                last = (n0 == NB - 1)
                for nt in range(n0 * 4, (n0 + 1) * 4):
                    ot = work.tile([128, 2, 512], dt.bfloat16, tag="o")
                    for d0 in range(NB):
                        po = psA.tile([128, 512], f32, tag="acc",
                                      name=f"po{nt}_{d0}")
                        for kk in range(HD // 128):
                            nc.tensor.matmul(
                                po[:],
                                ct_sb[:, kk, nt * 128:(nt + 1) * 128],
                                wo_sb[:, kk, d0 * 512:(d0 + 1) * 512],
                                start=(kk == 0),
                                stop=(kk == HD // 128 - 1),
                            )
                        if last and d0 == 1:
                            nc.scalar.copy(ot[:, d0, :], po[:])
                        else:
                            nc.vector.tensor_copy(ot[:, d0, :], po[:])
                        eng = (nc.scalar if (last and d0 == 1) else nc.sync)
                        eng.dma_start(
                            out[nt * 128:(nt + 1) * 128,
                                d0 * 512:(d0 + 1) * 512],
                            ot[:, d0, :])

    nc.finalize()
    return nc


def _get_nc(H_c: int, n_kept: int, mode: str, half_last: bool):
    key = (H_c, n_kept, mode, half_last)
    if key not in _cache:
        _cache[key] = _build(H_c, HEADS / n_kept, mode, half_last)
    return _cache[key]


def _prep_inputs(x1, x2, spd, head_keep, Wq, Wkv, Wo, mode="bf16"):
    """Slice/transpose/pad host-side into per-core input maps."""
    import ml_dtypes

    ndt = np.float32 if mode in ("f32", "f32r") else ml_dtypes.bfloat16
    kept = [int(i) for i in np.nonzero(head_keep)[0]]
    n_kept = len(kept)
    half_last = (n_kept % 2 == 1)
    if not half_last:
        H_c = n_kept // 2
        if H_c % 2:
            H_c += 1
        groups = [kept[:H_c], kept[H_c:]]
    else:
        # odd count: both cores of a pair share the last kept head, each
        # computing one n-half of it (local column order differs per core)
        K = (n_kept - 1) // 2
        shared = kept[-1]
        H_c = K + 1
        pad = []
        if H_c % 2:
            H_c += 1
            pad = [None]
        groups = [kept[:K] + pad + [shared], kept[K:2 * K] + pad + [shared]]

    Wk_full, Wv_full = Wkv[:, :INNER], Wkv[:, INNER:]

    in_maps = []
    for b in range(B):
        xqT = np.ascontiguousarray(
            x1[b].T.reshape(KT, 128, N).transpose(1, 0, 2)).astype(ndt)
        xkT = np.ascontiguousarray(
            x2[b].T.reshape(KT, 128, N).transpose(1, 0, 2)).astype(ndt)
        for g in range(2):
            heads = groups[g]
            swap = half_last and g == 1  # local n0=0 <-> global half 1
            xq_g = xqT
            if swap:
                xq_g = np.ascontiguousarray(
                    np.concatenate([xqT[:, :, 512:], xqT[:, :, :512]], axis=2))
            HD = H_c * DIM_HEAD
            wq_c = np.zeros((DIM, HD), np.float32)
            wk_c = np.zeros((DIM, HD), np.float32)
            wv_c = np.zeros((DIM, HD), np.float32)
            wo_c = np.zeros((HD, DIM), np.float32)
            # exp(spd) multiplies exp(qk); absent slots get 1.0 (identity)
            spd_c = np.ones((H_c, NB, 128, MT, 512), ndt)
            for i, h in enumerate(heads):
                if h is None:
                    continue
                sl = slice(i * DIM_HEAD, (i + 1) * DIM_HEAD)
                hs = slice(h * DIM_HEAD, (h + 1) * DIM_HEAD)
                wq_c[:, sl] = Wq[:, hs] * SCALE
                wk_c[:, sl] = Wk_full[:, hs]
                wv_c[:, sl] = Wv_full[:, hs]
                wo_c[sl, :] = Wo[hs, :]
                # exp(spd[b,h]).T -> [n-block, partition, m-tile, n] image,
                # n-blocks in the core's LOCAL column order
                im = (np.exp(spd[b, h].T).reshape(MT, 128, NB, 512)
                      .transpose(2, 1, 0, 3)).astype(ndt)
                spd_c[i] = im[::-1] if swap else im
            in_maps.append({
                "xq": xq_g,
                "xk": xkT,
                "wq": np.ascontiguousarray(
                    wq_c.reshape(KT, 128, HD).transpose(1, 0, 2)).astype(ndt),
                "wk": np.ascontiguousarray(
                    wk_c.reshape(KT, 128, HD).transpose(1, 0, 2)).astype(ndt),
                "wv": np.ascontiguousarray(
                    wv_c.reshape(KT, 128, HD).transpose(1, 0, 2)).astype(ndt),
                "wo": np.ascontiguousarray(
                    wo_c.reshape(HD // 128, 128, DIM).transpose(1, 0, 2)).astype(ndt),
                "spd": spd_c,
            })
    return in_maps, n_kept, H_c


def _run(nc, in_maps, trace=False, tmpdir=None):
    from concourse.bass_utils import run_bass_kernel_spmd

    return run_bass_kernel_spmd(
        nc, in_maps, core_ids=list(range(NCORES)), trace=trace, tmpdir=tmpdir
    )


def kernel(x1, x2, spd, head_keep, Wq, Wkv, Wo, bo, _trace=False, _tmpdir=None):
    x1 = np.asarray(x1, np.float32)
    x2 = np.asarray(x2, np.float32)
    spd = np.asarray(spd, np.float32)
    head_keep = np.asarray(head_keep)
    n_kept = int(head_keep.astype(np.int64).sum())
    if n_kept == 0:
        # reference: 16/0 = inf, 0*inf = nan everywhere
        return np.full((B, N, DIM), np.nan, np.float32)

    mode = os.environ.get("KERNEL_DTYPE", "bf16")
    in_maps, n_kept, H_c = _prep_inputs(
        x1, x2, spd, head_keep, Wq, Wkv, Wo, mode)
    half_last = (n_kept % 2 == 1)
    nc = _get_nc(H_c, n_kept, mode, half_last)
    res = _run(nc, in_maps, trace=_trace, tmpdir=_tmpdir)

    out = np.empty((B, N, DIM), np.float32)
    bo32 = np.asarray(bo, np.float32)
    for b in range(B):
        o0 = res.results[2 * b]["out"].astype(np.float32)
        o1 = res.results[2 * b + 1]["out"].astype(np.float32)
        if half_last:
            o1 = np.concatenate([o1[512:], o1[:512]], axis=0)
        out[b] = o0 + o1 + bo32
    kernel._last_results = res
    return out

